# revision 10
# baseline (speedup 1.0000x reference)
"""AnchorLoss Trainium2 kernel.

loss = sum_{b,i,j: mask[b,i,j]==1} (1 - exp(-|z_i - z_j|^2 / 10)),  z = embedding + abs_coords

Data-parallel over batch B=8, one batch per NeuronCore. Per core: device
prep (z = e + a, |z|^2, bf16 hi/lo splits), stream the [2048, 2048] int32
mask in 16 row blocks on the Sync HWDGE ring (blocks 3+ deferred behind
the small prep placement DMAs so they are not starved on the shared SDMA
engines), per 1024-col chunk K=14 bf16 matmul -> PSUM d2, ScalarE exp
(scale=-0.1) in place, VectorE fused (E - 1) * mask with per-partition
accumulate. The last block's mask DMA is split into column halves so the
final chunk's compute trails the stream by ~1us, and partial sums for
blocks 0-14 ship while block 15 streams. Host sums and negates.
"""
import numpy as np
import sys

for _p in ("/opt/trn_rl_repo", "/root/.axon_site/_ro/trn_rl_repo"):
    if _p not in sys.path:
        sys.path.append(_p)

N = 2048
B = 8

_CACHED = None


def _build(n=N):
    from concourse import bacc, mybir, tile
    from concourse.tile import add_dep_helper

    f32 = mybir.dt.float32
    i32 = mybir.dt.int32
    bf16 = mybir.dt.bfloat16
    AF = mybir.ActivationFunctionType
    ALU = mybir.AluOpType

    nb = n // 128
    cw = min(n, 1024)
    nch = n // cw
    nj = cw // 512

    G = 8
    w = n // G
    nc = bacc.Bacc()
    ea_in = nc.declare_dram_parameter("ea", [2 * G, 2 * w], f32, isOutput=False)
    m_in = nc.declare_dram_parameter("m", [n, n], i32, isOutput=False)
    ncols = nb * nch + (n // 512 - nch)
    out = nc.declare_dram_parameter("out", [128, ncols], f32, isOutput=True)

    with tile.TileContext(nc) as tc:
        with (
            tc.tile_pool(name="singles", bufs=1) as singles,
            tc.tile_pool(name="maskp", bufs=10) as maskp,
            tc.tile_pool(name="psum", bufs=4, space="PSUM") as psump,
        ):
            dummy = singles.tile([1, 8], f32)
            nc.gpsimd.memset(dummy[:], 0.0)
            nc.scalar.activation(dummy[:], dummy[:], AF.Exp)

            ea = singles.tile([2 * G, 2 * w], f32)
            nc.sync.dma_start(ea[:], ea_in[:])
            zt = singles.tile([2 * G, w], f32)
            nc.vector.tensor_tensor(zt[:], ea[:, 0:w], ea[:, w:2 * w], ALU.add)
            sq = singles.tile([2 * G, w], f32)
            nc.vector.tensor_tensor(sq[:], zt[:], zt[:], ALU.mult)

            zh = singles.tile([2 * G, w], bf16)
            zl = singles.tile([2 * G, w], bf16)
            sqh = singles.tile([2 * G, w], bf16)
            sql = singles.tile([2 * G, w], bf16)
            m2zh = singles.tile([2 * G, w], bf16)
            m2zl = singles.tile([2 * G, w], bf16)
            nc.scalar.activation(zh[:], zt[:], AF.Copy)
            nc.vector.tensor_tensor(zl[:], zt[:], zh[:], ALU.subtract)
            nc.scalar.activation(sqh[:], sq[:], AF.Copy)
            nc.vector.tensor_tensor(sql[:], sq[:], sqh[:], ALU.subtract)
            nc.vector.tensor_scalar_mul(m2zh[:], zh[:], -2.0)
            nc.vector.tensor_scalar_mul(m2zl[:], zl[:], -2.0)
            ones4 = singles.tile([4, n], bf16)
            nc.vector.memset(ones4[:], 1.0)

            zcol = singles.tile([14, n], bf16)
            nc.gpsimd.dma_start(zcol[0:2, :], sqh[:])
            nc.gpsimd.dma_start(zcol[2:4, :], sql[:])
            nc.gpsimd.dma_start(zcol[4:8, :], ones4[:])
            nc.gpsimd.dma_start(zcol[8:10, :], m2zh[:])
            nc.gpsimd.dma_start(zcol[10:12, :], m2zl[:])
            zcol_tail = nc.gpsimd.dma_start(zcol[12:14, :], m2zh[:])

            zrow = singles.tile([14, n], bf16)
            nc.scalar.dma_start(zrow[0:4, :], ones4[:])
            nc.scalar.dma_start(zrow[4:6, :], sqh[:])
            nc.scalar.dma_start(zrow[6:8, :], sql[:])
            nc.scalar.dma_start(zrow[8:10, :], zh[:])
            nc.scalar.dma_start(zrow[10:12, :], zh[:])
            zrow_tail = nc.scalar.dma_start(zrow[12:14, :], zl[:])

            acc = singles.tile([128, ncols], f32)

            acol = 0
            for ib in range(nb):
                mk = maskp.tile([128, n], i32)
                if ib < nb - 1:
                    mdmas = [nc.sync.dma_start(
                        mk[:], m_in[ib * 128:(ib + 1) * 128, :])]
                else:
                    # final block in column halves: the first 1024-chunk's
                    # compute can start while the second half streams
                    mdmas = [
                        nc.sync.dma_start(
                            mk[:, k * cw:(k + 1) * cw],
                            m_in[ib * 128:(ib + 1) * 128, k * cw:(k + 1) * cw],
                        )
                        for k in range(2)
                    ]
                if ib >= 3:
                    for mdma in mdmas:
                        add_dep_helper(mdma.ins, zcol_tail.ins,
                                       reason="defer mask stream behind prep")
                        add_dep_helper(mdma.ins, zrow_tail.ins,
                                       reason="defer mask stream behind prep")
                widths = [512] * (n // 512) if ib == 0 else [cw] * nch
                col = 0
                for wch in widths:
                    ps = psump.tile([128, wch], f32)
                    for jc in range(wch // 512):
                        c0 = col + jc * 512
                        nc.tensor.matmul(
                            ps[:, jc * 512:(jc + 1) * 512],
                            zrow[:, ib * 128:(ib + 1) * 128],
                            zcol[:, c0:c0 + 512],
                            start=True,
                            stop=True,
                        )
                    nc.scalar.activation(ps[:], ps[:], AF.Exp, scale=-0.1)
                    nc.vector.scalar_tensor_tensor(
                        ps[:], ps[:], 1.0, mk[:, col:col + wch],
                        op0=ALU.subtract, op1=ALU.mult,
                        accum_out=acc[:, acol:acol + 1],
                    )
                    col += wch
                    acol += 1
                if ib == nb - 2:
                    # cols for blocks 0-14 are final: ship them while the
                    # last block is still streaming
                    nc.scalar.dma_start(out[:, 0:acol], acc[:, 0:acol])
            nc.scalar.dma_start(out[:, ncols - 2:], acc[:, ncols - 2:])
    nc.compile()
    return nc


def _get_graph():
    global _CACHED
    if _CACHED is None:
        _CACHED = _build()
    return _CACHED


def _pack_ea(e, a, n, G=8):
    w = n // G
    ea = np.empty((2 * G, 2 * w), dtype=np.float32)
    for d in range(2):
        ea[d * G:(d + 1) * G, :w] = e[:, d].reshape(G, w)
        ea[d * G:(d + 1) * G, w:] = a[:, d].reshape(G, w)
    return ea


def kernel(embedding, abs_coords, patch_mask, _trace=False, _trace_kwargs=None):
    from concourse.bass_utils import run_bass_kernel_spmd

    nc = _get_graph()
    in_maps = [
        {
            "ea": _pack_ea(embedding[b], abs_coords[b], N),
            "m": np.ascontiguousarray(patch_mask[b], dtype=np.int32),
        }
        for b in range(B)
    ]
    kw = {}
    if _trace:
        kw = dict(trace=True, **(_trace_kwargs or {}))
    res = None
    last_err = None
    for _attempt in range(3):
        try:
            res = run_bass_kernel_spmd(nc, in_maps, core_ids=list(range(B)), **kw)
            total = -sum(
                float(np.sum(np.asarray(r["out"]), dtype=np.float64))
                for r in res.results
            )
            break
        except Exception as err:
            last_err = err
            res = None
    if res is None:
        raise last_err
    out = np.float32(total)
    if _trace:
        return out, res
    return out


# revision 11
# speedup vs baseline: 1.0537x; 1.0537x over previous
"""AnchorLoss Trainium2 kernel.

loss = sum_{b,i,j: mask[b,i,j]==1} (1 - exp(-|z_i - z_j|^2 / 10)),  z = embedding + abs_coords

Sharding: data-parallel over batch B=8, one batch per NeuronCore. Each core:
  - device-side prep: z = e + a, r = |z|^2, bf16 hi/lo splits (pseudo-fp32),
  - streams its [2048, 2048] int32 mask in 16 row-blocks of [128, 2048],
  - per 1024-col chunk: K=14 bf16 matmul -> PSUM = d2 (hi/lo expansion),
    ScalarE exp with scale=-0.1, VectorE fused (E - 1) * mask with
    per-partition accumulate,
  - returns [128, 32] partial sums; host sums and negates.

The host passes e/a stacked+transposed+folded as one [16, N/4] array
(layout only, zero flops): row d*8+g holds [e_d chunk g | a_d chunk g],
so prep ops run 16-partition-wide (~0.4us each) and the coordinate load
is a single small DMA.

Notes from the optimization campaign (trace-verified on HW):
  - the mask stream runs at ~380-390 GB/s on the Sync HWDGE ring, at the
    HBM-per-core limit; the defer of mask tiles 3+ behind the placement
    queues is load-bearing (small placement DMAs otherwise starve behind
    the queued mask stream on the shared SDMA engines, delaying the first
    matmul by ~10us),
  - 1024-wide ACT/DVE chunking is load-bearing (it hides the cross-engine
    handoff latency; 2048-wide ops expose it and cost ~1us/block),
  - raw-bacc variants (manual semaphores, no TileContext) were slower:
    per-DMA-sound semaphores plus exposed stage handoffs cost more than
    Tile's scheduling slop, and the ~7us teardown is mostly a fixed
    postamble either way.
"""
import numpy as np
import sys

for _p in ("/opt/trn_rl_repo", "/root/.axon_site/_ro/trn_rl_repo"):
    if _p not in sys.path:
        sys.path.append(_p)

N = 2048
B = 8

_CACHED = None


def _build(n=N):
    from concourse import bacc, mybir, tile
    from concourse.tile import add_dep_helper

    f32 = mybir.dt.float32
    i32 = mybir.dt.int32
    bf16 = mybir.dt.bfloat16
    AF = mybir.ActivationFunctionType
    ALU = mybir.AluOpType

    nb = n // 128
    cw = min(n, 1024)
    nch = n // cw
    nj = cw // 512

    G = 8
    w = n // G
    nc = bacc.Bacc()
    ea_in = nc.declare_dram_parameter("ea", [2 * G, 2 * w], f32, isOutput=False)
    m_in = nc.declare_dram_parameter("m", [n, n], i32, isOutput=False)
    ncols = nb * nch + (n // 512 - nch)
    out = nc.declare_dram_parameter("out", [128, ncols], f32, isOutput=True)

    with tile.TileContext(nc) as tc:
        with (
            tc.tile_pool(name="singles", bufs=1) as singles,
            tc.tile_pool(name="maskp", bufs=10) as maskp,
            tc.tile_pool(name="psum", bufs=4, space="PSUM") as psump,
        ):
            dummy = singles.tile([1, 8], f32)
            nc.gpsimd.memset(dummy[:], 0.0)
            nc.scalar.activation(dummy[:], dummy[:], AF.Exp)

            ea = singles.tile([2 * G, 2 * w], f32)
            nc.sync.dma_start(ea[:], ea_in[:])
            zt = singles.tile([2 * G, w], f32)
            nc.vector.tensor_tensor(zt[:], ea[:, 0:w], ea[:, w:2 * w], ALU.add)
            sq = singles.tile([2 * G, w], f32)
            nc.vector.tensor_tensor(sq[:], zt[:], zt[:], ALU.mult)

            zh = singles.tile([2 * G, w], bf16)
            zl = singles.tile([2 * G, w], bf16)
            sqh = singles.tile([2 * G, w], bf16)
            sql = singles.tile([2 * G, w], bf16)
            m2zh = singles.tile([2 * G, w], bf16)
            m2zl = singles.tile([2 * G, w], bf16)
            nc.scalar.activation(zh[:], zt[:], AF.Copy)
            nc.vector.tensor_tensor(zl[:], zt[:], zh[:], ALU.subtract)
            nc.scalar.activation(sqh[:], sq[:], AF.Copy)
            nc.vector.tensor_tensor(sql[:], sq[:], sqh[:], ALU.subtract)
            nc.vector.tensor_scalar_mul(m2zh[:], zh[:], -2.0)
            nc.vector.tensor_scalar_mul(m2zl[:], zl[:], -2.0)
            ones4 = singles.tile([4, n], bf16)
            nc.vector.memset(ones4[:], 1.0)

            zcol = singles.tile([14, n], bf16)
            nc.gpsimd.dma_start(zcol[0:2, :], sqh[:])
            nc.gpsimd.dma_start(zcol[2:4, :], sql[:])
            nc.gpsimd.dma_start(zcol[4:8, :], ones4[:])
            nc.gpsimd.dma_start(zcol[8:10, :], m2zh[:])
            nc.gpsimd.dma_start(zcol[10:12, :], m2zl[:])
            zcol_tail = nc.gpsimd.dma_start(zcol[12:14, :], m2zh[:])

            zrow = singles.tile([14, n], bf16)
            nc.scalar.dma_start(zrow[0:4, :], ones4[:])
            nc.scalar.dma_start(zrow[4:6, :], sqh[:])
            nc.scalar.dma_start(zrow[6:8, :], sql[:])
            nc.scalar.dma_start(zrow[8:10, :], zh[:])
            nc.scalar.dma_start(zrow[10:12, :], zh[:])
            zrow_tail = nc.scalar.dma_start(zrow[12:14, :], zl[:])

            acc = singles.tile([128, ncols], f32)

            acol = 0
            for ib in range(nb):
                mk = maskp.tile([128, n], i32)
                mdma = nc.sync.dma_start(mk[:], m_in[ib * 128:(ib + 1) * 128, :])
                if ib >= 3:
                    add_dep_helper(mdma.ins, zcol_tail.ins,
                                   reason="defer mask stream behind prep")
                    add_dep_helper(mdma.ins, zrow_tail.ins,
                                   reason="defer mask stream behind prep")
                widths = [512] * (n // 512) if ib == 0 else [cw] * nch
                col = 0
                for wch in widths:
                    ps = psump.tile([128, wch], f32)
                    for jc in range(wch // 512):
                        c0 = col + jc * 512
                        nc.tensor.matmul(
                            ps[:, jc * 512:(jc + 1) * 512],
                            zrow[:, ib * 128:(ib + 1) * 128],
                            zcol[:, c0:c0 + 512],
                            start=True,
                            stop=True,
                        )
                    nc.scalar.activation(ps[:], ps[:], AF.Exp, scale=-0.1)
                    nc.vector.scalar_tensor_tensor(
                        ps[:], ps[:], 1.0, mk[:, col:col + wch],
                        op0=ALU.subtract, op1=ALU.mult,
                        accum_out=acc[:, acol:acol + 1],
                    )
                    col += wch
                    acol += 1
            h1 = 3 * ncols // 4
            nc.scalar.dma_start(out[:, 0:h1], acc[:, 0:h1])
            nc.scalar.dma_start(out[:, h1:], acc[:, h1:])
    nc.compile()
    return nc


def _get_graph():
    global _CACHED
    if _CACHED is None:
        _CACHED = _build()
    return _CACHED


def _pack_ea(e, a, n, G=8):
    w = n // G
    ea = np.empty((2 * G, 2 * w), dtype=np.float32)
    for d in range(2):
        ea[d * G:(d + 1) * G, :w] = e[:, d].reshape(G, w)
        ea[d * G:(d + 1) * G, w:] = a[:, d].reshape(G, w)
    return ea


def kernel(embedding, abs_coords, patch_mask, _trace=False, _trace_kwargs=None):
    from concourse.bass_utils import run_bass_kernel_spmd

    nc = _get_graph()
    in_maps = [
        {
            "ea": _pack_ea(embedding[b], abs_coords[b], N),
            "m": np.ascontiguousarray(patch_mask[b], dtype=np.int32),
        }
        for b in range(B)
    ]
    kw = {}
    if _trace:
        kw = dict(trace=True, **(_trace_kwargs or {}))
    res = None
    last_err = None
    for _attempt in range(3):
        try:
            res = run_bass_kernel_spmd(nc, in_maps, core_ids=list(range(B)), **kw)
            total = -sum(
                float(np.sum(np.asarray(r["out"]), dtype=np.float64))
                for r in res.results
            )
            break
        except Exception as err:
            last_err = err
            res = None
    if res is None:
        raise last_err
    out = np.float32(total)
    if _trace:
        return out, res
    return out
